# revision 17
# baseline (speedup 1.0000x reference)
"""Trainium2 Bass kernel for nn_MultiHeadAttention_73607149519012.

MHA: B=8, S=1024, D=1024, H=16 heads, depth=64, fp32 in/out.
Sharding: data-parallel over batch -- one batch element per NeuronCore (8 cores).

Per-core computation (batch b), all layouts chosen so every matmul contracts
over the partition dimension with NO on-device transposes (host pre-transposes
the three activation inputs). Matmuls run in bf16 with fp32 PSUM accumulation.

  QT = (Wq^T x^T)           : lhsT=Wq (dm,dout) tiles, rhs=x^T (dm,s)  -> (dout, s)
  KT = (Wk^T x^T)           : same                                      -> (dout, s)
  V  = (x Wv)               : lhsT=x^T (dm,s) tiles,  rhs=Wv (dm,dv)   -> (s, dv)
  logitsT[kpos,q] per head  : lhsT=KhT (d,kpos), rhs=QhT (d,q)  (K=64, two heads
                              packed in PE row-groups 0-63 / 64-127)
  pT = exp(logitsT * 1/8)   : ScalarE; the additive mask is applied
                              multiplicatively to V rows (exp(l+m) = exp(l)*exp(m))
  ctxT'[d+1,q] per head     : lhsT=[Vh | mask'] (kpos,65), rhs=pT (kpos,q)
                              row 64 accumulates the softmax row-sums
  ctxT = ctxT'[0:64] * (1/rowsum broadcast)   (GpSimd partition_broadcast + DVE)
  outA = x Wo[:D] + bo      : accumulated DURING attention (PE is ACT-bound there)
  out  = outA + ctxT^T Wo[D:]

Phase structure (PSUM banks):
  phase 1: projections, 8 held psum groups, k-tiles streamed outer  (8 banks)
  phase 2: attention (4 logits + 2 PV banks) + outA groups (2 banks)
  phase 3: ctx half of the output projection (8 banks)
"""

import os

import numpy as np

import concourse.bass as bass
import concourse.mybir as mybir
import concourse.tile as tile
from concourse import bacc
from concourse.bass_utils import run_bass_kernel_spmd

F32 = mybir.dt.float32
AF = mybir.ActivationFunctionType

D = 1024          # model dim
S = 1024          # sequence length
H = 16            # heads
DEP = 64          # head dim
B = 8             # batch == n cores
NT = 8            # 128-partition tiles per 1024 dim
SCALE = 1.0 / 8.0  # 1/sqrt(DEP)

# matmul input dtype. bf16 runs the PE at 1 cycle/row with fp32 PSUM
# accumulation and halves DMA traffic; float32 is exact but 4 cycles/row.
_MM_DT_NAME = os.environ.get("MHA_MM_DT", "bf16")
MM_DT = {"bf16": mybir.dt.bfloat16, "f32": mybir.dt.float32}[_MM_DT_NAME]
MM_NP = mybir.dt.np(MM_DT)

LAST_EXEC_NS = None


def _mm(nc, out, lhsT, rhs, start, stop):
    nc.tensor.matmul(out, lhsT, rhs, start=start, stop=stop)


def build_nc(phases=(1, 2, 3)):
    nc = bacc.Bacc(None, target_bir_lowering=False)

    xqT_d = nc.dram_tensor("xqT", [D, S], MM_DT, kind="ExternalInput")
    xkT_d = nc.dram_tensor("xkT", [D, S], MM_DT, kind="ExternalInput")
    xvT_d = nc.dram_tensor("xvT", [D, S], MM_DT, kind="ExternalInput")
    wq_d = nc.dram_tensor("wq", [D, D], MM_DT, kind="ExternalInput")
    wk_d = nc.dram_tensor("wk", [D, D], MM_DT, kind="ExternalInput")
    wv_d = nc.dram_tensor("wv", [D, D], MM_DT, kind="ExternalInput")
    wo_d = nc.dram_tensor("wo", [2 * D, D], MM_DT, kind="ExternalInput")
    bq_d = nc.dram_tensor("bq", [128, NT], F32, kind="ExternalInput")
    bk_d = nc.dram_tensor("bk", [128, NT], F32, kind="ExternalInput")
    bv_d = nc.dram_tensor("bv", [128, D], MM_DT, kind="ExternalInput")
    bo_d = nc.dram_tensor("bo", [128, D], F32, kind="ExternalInput")
    # multiplicative mask exp(-1e9 * mask), laid out [kpos%128, kpos//128]
    mp_d = nc.dram_tensor("mprime", [128, NT], F32, kind="ExternalInput")
    out_d = nc.dram_tensor("out", [S, D], F32, kind="ExternalOutput")

    with tile.TileContext(nc) as tc:
        with (
            tc.tile_pool(name="const", bufs=1) as cp,
            tc.tile_pool(name="qtp", bufs=1) as qtp,
            tc.tile_pool(name="ktp", bufs=1) as ktp,
            tc.tile_pool(name="vp", bufs=1) as vp,
        ):
            bq_sb = cp.tile([128, NT], F32, name="bq_sb")
            nc.sync.dma_start(bq_sb, bq_d[:, :])
            bk_sb = cp.tile([128, NT], F32, name="bk_sb")
            nc.sync.dma_start(bk_sb, bk_d[:, :])
            bv_sb = cp.tile([128, D], MM_DT, name="bv_sb")
            nc.sync.dma_start(bv_sb, bv_d[:, :])
            bo_sb = cp.tile([128, D], F32, name="bo_sb")
            nc.sync.dma_start(bo_sb, bo_d[:, :])
            mp_sb = cp.tile([128, NT], F32, name="mp_sb")
            nc.sync.dma_start(mp_sb, mp_d[:, :])

            # Residents: QT, KT (dout-major) and V65 (s-major, 65 cols/head:
            # 64 of V plus one mask' column that accumulates the softmax sum).
            qt_sb = [qtp.tile([128, S], MM_DT, name=f"qt{i}", tag=f"qt{i}") for i in range(NT)]
            kt_sb = [ktp.tile([128, S], MM_DT, name=f"ktile{i}", tag=f"ktile{i}") for i in range(NT)]
            v65_sb = [vp.tile([128, H, 65], MM_DT, name=f"v65_{i}", tag=f"v65_{i}") for i in range(NT)]

            # ================= Phase 1: projections =================
            # k-tiles outer, 8 psum groups held -> the first matmul only waits
            # for the first w/x tile pair instead of the whole weight matrix.
            with (
                tc.tile_pool(name="wpool", bufs=1) as wpool,
                tc.tile_pool(name="xpool", bufs=1) as xpool,
                tc.tile_pool(name="pps", bufs=1, space="PSUM") as pps,
            ):
                for w_d, x_d, kind in ((wq_d, xqT_d, "q"), (wk_d, xkT_d, "k"), (wv_d, xvT_d, "v")):
                    if 1 not in phases:
                        break
                    w_sb = []
                    x_sb = []
                    for kt in range(NT):
                        wt = wpool.tile([128, D], MM_DT, name=f"w_{kind}_{kt}", tag=f"w{kt}")
                        nc.sync.dma_start(wt, w_d[kt * 128:(kt + 1) * 128, :])
                        w_sb.append(wt)
                        xt = xpool.tile([128, S], MM_DT, name=f"x_{kind}_{kt}", tag=f"x{kt}")
                        nc.sync.dma_start(xt, x_d[kt * 128:(kt + 1) * 128, :])
                        x_sb.append(xt)

                    for half in range(2):
                        groups = [(a, c) for a in range(half * 4, half * 4 + 4) for c in range(2)]
                        psums = {}
                        for i, g in enumerate(groups):
                            psums[g] = pps.tile([128, 512], F32, name=f"ps_{kind}_{g[0]}_{g[1]}", tag=f"pp{i}")
                        for kt in range(NT):
                            for (a, c) in groups:
                                if kind in ("q", "k"):
                                    lhsT = w_sb[kt][:, a * 128:(a + 1) * 128]
                                    rhs = x_sb[kt][:, c * 512:(c + 1) * 512]
                                else:
                                    lhsT = x_sb[kt][:, a * 128:(a + 1) * 128]
                                    rhs = w_sb[kt][:, c * 512:(c + 1) * 512]
                                _mm(nc, psums[(a, c)], lhsT, rhs,
                                    start=(kt == 0), stop=(kt == NT - 1))
                        if kind in ("q", "k"):
                            dst = qt_sb if kind == "q" else kt_sb
                            bias = bq_sb if kind == "q" else bk_sb
                            for (dt, sc) in groups:
                                nc.vector.tensor_scalar_add(
                                    dst[dt][:, sc * 512:(sc + 1) * 512], psums[(dt, sc)],
                                    bias[:, dt:dt + 1])
                        else:
                            for st in range(half * 4, half * 4 + 4):
                                nc.vector.memset(v65_sb[st][:, :, 64:65], 1.0)
                            for (st, c) in groups:
                                nc.vector.tensor_add(
                                    v65_sb[st][:, c * 8:(c + 1) * 8, 0:64],
                                    psums[(st, c)].rearrange("p (h e) -> p h e", e=64),
                                    bv_sb[:, c * 512:(c + 1) * 512].rearrange("p (h e) -> p h e", e=64))
                            # mask': scale V rows (and the ones col) by exp(-1e9*mask[kpos])
                            for st in range(half * 4, half * 4 + 4):
                                nc.vector.tensor_scalar_mul(
                                    v65_sb[st][:, :, :], v65_sb[st][:, :, :], mp_sb[:, st:st + 1])

            # ============ Phases 2+3: attention + output projection ============
            with (
                tc.tile_pool(name="ctxp", bufs=1) as ctxp,
                tc.tile_pool(name="xq2p", bufs=1) as xq2p,
                tc.tile_pool(name="wopA", bufs=1) as wopA,
                tc.tile_pool(name="oap", bufs=1) as oap,
                tc.tile_pool(name="ptp", bufs=2) as ptp,
                tc.tile_pool(name="rp", bufs=2) as rp,
            ):
                ctx_sb = [ctxp.tile([128, S], MM_DT, name=f"ctx{i}", tag=f"ctx{i}") for i in range(NT)]
                oa_sb = [oap.tile([128, D], F32, name=f"oa{i}", tag=f"oa{i}") for i in range(NT)]

                # x^T reload + Wo[:D] tiles: both consumed by outA groups that
                # run interleaved with attention (PE is ACT-bound there).
                xq2_sb = []
                woA_sb = {}
                if 2 in phases:
                    for kt in range(NT):
                        xt2 = xq2p.tile([128, S], MM_DT, name=f"xq2_{kt}", tag=f"xq2_{kt}")
                        nc.sync.dma_start(xt2, xqT_d[kt * 128:(kt + 1) * 128, :])
                        xq2_sb.append(xt2)
                    for kt2 in range(NT):
                        for dc in range(2):
                            wt = wopA.tile([128, 512], MM_DT, name=f"woA_{kt2}_{dc}", tag=f"woA_{kt2}_{dc}")
                            nc.sync.dma_start(wt, wo_d[kt2 * 128:(kt2 + 1) * 128, dc * 512:(dc + 1) * 512])
                            woA_sb[(kt2, dc)] = wt

                with (
                    tc.tile_pool(name="lpsp", bufs=1, space="PSUM") as lpsp,
                    tc.tile_pool(name="pvp", bufs=2, space="PSUM") as pvp,
                    tc.tile_pool(name="oaps", bufs=2, space="PSUM") as oaps,
                ):
                    for dt in range(NT) if 2 in phases else []:
                        pair = ((2 * dt, 0), (2 * dt + 1, 64))
                        for qc in range(2):
                            pts = {}
                            for h, base in pair:
                                pts[h] = ptp.tile([128, NT, 512], MM_DT, name=f"pt_{h}_{qc}", tag=f"pt{h % 2}")
                            for g in range(4):
                                lps = {}
                                for h, base in pair:
                                    lps[h] = lpsp.tile([128, 1024], F32, name=f"lps_{h}_{qc}_{g}", tag=f"lps{h % 2}")
                                for j in range(2):
                                    kt = 2 * g + j
                                    for h, base in pair:
                                        _mm(nc, lps[h][:, j * 512:(j + 1) * 512],
                                            kt_sb[dt][base:base + 64, kt * 128:(kt + 1) * 128],
                                            qt_sb[dt][base:base + 64, qc * 512:(qc + 1) * 512],
                                            start=True, stop=True)
                                for h, base in pair:
                                    nc.scalar.activation(
                                        pts[h][:, 2 * g:2 * g + 2, :],
                                        lps[h].rearrange("p (a c) -> p a c", a=2),
                                        AF.Exp, scale=SCALE)
                            for h, base in pair:
                                pv = pvp.tile([65, 512], F32, name=f"pv_{h}_{qc}", tag="pv")
                                for kt in range(NT):
                                    _mm(nc, pv, v65_sb[kt][:, h, :], pts[h][:, kt, :],
                                        start=(kt == 0), stop=(kt == NT - 1))
                                recip = rp.tile([1, 512], F32, name=f"recip_{h}_{qc}", tag="recip")
                                nc.vector.reciprocal(recip, pv[64:65, :])
                                rbc = rp.tile([64, 512], F32, name=f"rbc_{h}_{qc}", tag="rbc")
                                nc.gpsimd.partition_broadcast(rbc, recip, channels=64)
                                nc.vector.tensor_mul(
                                    ctx_sb[dt][base:base + 64, qc * 512:(qc + 1) * 512],
                                    pv[0:64, :], rbc)

                            # one outA group (st, dc) per (dt, qc) iteration:
                            # outA[st, dc] = x Wo_top + bo, using PE slack in the
                            # ACT-bound attention window
                            idx = dt * 2 + qc
                            st, dc = idx // 2, idx % 2
                            psA = oaps.tile([128, 512], F32, name=f"psA_{st}_{dc}", tag="oaps")
                            for kt2 in range(NT):
                                _mm(nc, psA,
                                    xq2_sb[kt2][:, st * 128:(st + 1) * 128],
                                    woA_sb[(kt2, dc)],
                                    start=(kt2 == 0), stop=(kt2 == NT - 1))
                            nc.vector.tensor_add(
                                oa_sb[st][:, dc * 512:(dc + 1) * 512], psA,
                                bo_sb[:, dc * 512:(dc + 1) * 512])

                # ========= Phase 3: ctx half of the output projection =========
                with (
                    tc.tile_pool(name="wopB", bufs=4) as wopB,
                    tc.tile_pool(name="outp", bufs=4) as outp,
                    tc.tile_pool(name="ops", bufs=1, space="PSUM") as ops,
                ):
                    for dc in range(2) if 3 in phases else []:
                        pso = [ops.tile([128, 512], F32, name=f"pso_{dc}_{st}", tag=f"pso{st}")
                               for st in range(NT)]
                        for kt2 in range(NT, 2 * NT):
                            wo_t = wopB.tile([128, 512], MM_DT, name=f"woB_{dc}_{kt2}", tag="wo")
                            nc.sync.dma_start(wo_t, wo_d[kt2 * 128:(kt2 + 1) * 128, dc * 512:(dc + 1) * 512])
                            for st in range(NT):
                                _mm(nc, pso[st],
                                    ctx_sb[kt2 - NT][:, st * 128:(st + 1) * 128],
                                    wo_t,
                                    start=(kt2 == NT), stop=(kt2 == 2 * NT - 1))
                        for st in range(NT):
                            ot = outp.tile([128, 512], F32, name=f"ot_{dc}_{st}", tag="ot")
                            nc.vector.tensor_add(ot, pso[st], oa_sb[st][:, dc * 512:(dc + 1) * 512])
                            nc.sync.dma_start(
                                out_d[st * 128:(st + 1) * 128, dc * 512:(dc + 1) * 512], ot)

    nc.finalize()
    return nc


_NC_CACHE = None


def _get_nc():
    global _NC_CACHE
    if _NC_CACHE is None:
        _NC_CACHE = build_nc()
    return _NC_CACHE


def kernel(**inputs):
    global LAST_EXEC_NS
    v = np.asarray(inputs["v"], np.float32)
    k = np.asarray(inputs["k"], np.float32)
    q_in = np.asarray(inputs["q_in"], np.float32)
    mask = np.asarray(inputs["mask"], np.float32)
    wq_w = np.asarray(inputs["wq_w"], np.float32)
    wq_b = np.asarray(inputs["wq_b"], np.float32)
    wk_w = np.asarray(inputs["wk_w"], np.float32)
    wk_b = np.asarray(inputs["wk_b"], np.float32)
    wv_w = np.asarray(inputs["wv_w"], np.float32)
    wv_b = np.asarray(inputs["wv_b"], np.float32)
    wo_w = np.asarray(inputs["wo_w"], np.float32)
    wo_b = np.asarray(inputs["wo_b"], np.float32)

    bq = np.ascontiguousarray(wq_b.reshape(NT, 128).T)
    bk = np.ascontiguousarray(wk_b.reshape(NT, 128).T)
    bv = np.ascontiguousarray(np.broadcast_to(wv_b, (128, D))).astype(MM_NP)
    bo = np.ascontiguousarray(np.broadcast_to(wo_b, (128, D)))
    wq_m = wq_w.astype(MM_NP)
    wk_m = wk_w.astype(MM_NP)
    wv_m = wv_w.astype(MM_NP)
    wo_m = wo_w.astype(MM_NP)

    in_maps = []
    for b in range(B):
        mcol = np.exp(np.float32(-1e9) * mask[b, 0, 0, :]).astype(np.float32)
        in_maps.append({
            "xqT": np.ascontiguousarray(q_in[b].T.astype(MM_NP)),
            "xkT": np.ascontiguousarray(k[b].T.astype(MM_NP)),
            "xvT": np.ascontiguousarray(v[b].T.astype(MM_NP)),
            "wq": wq_m, "wk": wk_m, "wv": wv_m, "wo": wo_m,
            "bq": bq, "bk": bk, "bv": bv, "bo": bo,
            "mprime": np.ascontiguousarray(mcol.reshape(NT, 128).T),
        })

    nc = _get_nc()
    trace = os.environ.get("MHA_TRACE", "0") == "1"
    res = run_bass_kernel_spmd(nc, in_maps, core_ids=list(range(B)), trace=trace)
    LAST_EXEC_NS = res.exec_time_ns
    return np.stack([r["out"] for r in res.results], axis=0)
